# revision 2
# baseline (speedup 1.0000x reference)
"""MixLinear int8-decomposition GEMM for Trainium2 (8 NeuronCores).

Reference semantics:
  out_mask[k]  = any(|x[:,k]| > 20)                      (outlier columns)
  w_base       = w * (1-mask);  s_c[n] = f16(max|w_base|/127) clamped
  q_w          = rint(w_base / s_c)
  s_r[m]       = max|x[m,:]|/127 clamped
  q_x          = rint(x / s_r)
  y            = (q_x @ q_w.T) * s_r * s_c + (x*mask)@(w*mask).T + bias

Implementation trick: both GEMMs are fused into ONE fp16 PE matmul.
For non-outlier columns the operands are the integer-valued q_x/q_w
(fp16 holds them exactly; fp32 PSUM accumulation is exact), for outlier
columns the operands are the UNROUNDED x/s_r and w/s_c, so that after the
(* s_r * s_c) epilogue those columns contribute x*w — the outlier GEMM.
Round-to-nearest-even is done with a per-column magic bias 1536*zmask[k]
(the fp16 +1536 trick rounds where zmask==1 and passes through where 0).

Sharding: column-parallel over N (out_features) across 8 cores; x replicated.
"""
import sys

sys.path.insert(0, '/opt/trn_rl_repo')

import numpy as np

import concourse.bass as bass
import concourse.mybir as mybir
from concourse.tile import TileContext
from concourse.vector_clock import ScopedClock, VectorClock

F16 = mybir.dt.float16
F32 = mybir.dt.float32
U16 = mybir.dt.uint16
ALU = mybir.AluOpType
ACTF = mybir.ActivationFunctionType

SIGMA = 20.0
MAGIC = 1536.0  # 1.5*2^10: fp16 round-to-nearest-even for |v| <= 127


class SplitDrainTileContext(TileContext):
    """This walrus build rejects >1 sem-wait on CTRL-type (Drain/NOP)
    instructions; pre-drain each clock lane with its own single-wait NOP and
    emit the final drain wait-free."""

    def _drain_and_barrier(self, tick_clock, wait_clock):
        gc = tick_clock.global_clock
        items = gc.items() if hasattr(gc, 'items') else [(None, gc)]
        for scope, vc in items:
            n = len(vc)
            for proc in range(n):
                t = vc[proc]
                if t > 0:
                    vec = [0] * n
                    vec[proc] = t
                    nop = self.nc.sync.nop()
                    wait_clock.add_sem_waits(
                        nop.ins, ScopedClock({scope: VectorClock(vec)}))
        self.nc.sync.drain()
        self.nc.all_engine_barrier()
        assert self.sems is not None
        popped = self.nc._tile_sem_poison_stack.pop()
        assert popped is self._sem_poison
        self.nc.clear_and_free_semaphores(list(self.sems.allocated().values()))
        self.nc.all_engine_barrier()


def _split_multi_waits(nc):
    """This walrus build rejects >1 semaphore wait per instruction (any
    struct). Hoist extra waits onto single-wait NOPs on the same engine,
    placed immediately before the instruction (engines execute their
    instructions in block order, so the stall semantics are identical)."""
    import bass_rust
    counter = [0]
    for f in nc.m.functions:
        for bb in f.blocks:
            new_list = []
            for inst in bb.instructions:
                si = inst.sync_info
                waits = list(si.on_wait) if si is not None else []
                if len(waits) > 1:
                    for wt in waits[:-1]:
                        counter[0] += 1
                        nop = mybir.InstNoOp(
                            name=f"I-waitsplit-{counter[0]}", ins=[], outs=[])
                        nop.engine = inst.engine
                        nop.sync_info = bass_rust.SyncInfo(
                            on_wait=[wt], on_update=[])
                        new_list.append(nop)
                    inst.sync_info = bass_rust.SyncInfo(
                        on_wait=[waits[-1]],
                        on_update=list(si.on_update) if si is not None else [])
                new_list.append(inst)
            bb.instructions = new_list


def build_bass(M, K, NL, split_waits=True):
    """One-core program: x[M,K]f16 (replicated), w[NL,K]f16 + bias[NL]f16
    (N-shard) -> y[M,NL]f16."""
    MT, KT, NW = M // 128, K // 128, NL // 128
    NB = K // 512  # count psum banks
    assert NL <= 512

    nc = bass.Bass()
    x = nc.declare_dram_parameter("x", [M, K], F16, isOutput=False)
    w = nc.declare_dram_parameter("w", [NL, K], F16, isOutput=False)
    bias = nc.declare_dram_parameter("b", [NL], F16, isOutput=False)
    ones16 = nc.declare_dram_parameter("ones16", [1, 128], F16, isOutput=False)
    ones32 = nc.declare_dram_parameter("ones32", [1, 128], F32, isOutput=False)
    onesm = nc.declare_dram_parameter("onesm", [128, 1], F16, isOutput=False)
    y = nc.declare_dram_parameter("y", [M, NL], F16, isOutput=True)

    scr_k16 = nc.dram_tensor("scr_k16", [K], F16)
    scr_nl = nc.dram_tensor("scr_nl", [2, NL], F32)

    with SplitDrainTileContext(nc) as tc:
        with tc.tile_pool(name="const", bufs=1) as pc:
            ones16_sb = pc.tile([1, 128], F16, tag="ones16")
            ones32_sb = pc.tile([1, 128], F32, tag="ones32")
            onesm_sb = pc.tile([128, 1], F16, tag="onesm")
            bias_sb = pc.tile([1, NL], F16, tag="bias")
            nc.gpsimd.dma_start(ones16_sb[:], ones16[:])
            nc.gpsimd.dma_start(ones32_sb[:], ones32[:])
            nc.gpsimd.dma_start(onesm_sb[:], onesm[:])
            nc.gpsimd.dma_start(bias_sb[0:1, :], bias[:])

            neg20 = pc.tile([128, 1], F32, tag="neg20")
            nc.vector.memset(neg20[:], -SIGMA)
            rowmax = pc.tile([128, MT], F32, tag="rowmax")
            s_r = pc.tile([128, MT], F32, tag="s_r")
            s_recip = pc.tile([128, MT], F32, tag="s_recip")
            zmask128 = pc.tile([128, K], F16, tag="zmask128")
            magicz128 = pc.tile([128, K], F16, tag="magicz128")
            scol128 = pc.tile([128, NL], F32, tag="scol128")
            screcipT = pc.tile([128, NL], F32, tag="screcipT")
            bias128 = pc.tile([128, NL], F32, tag="bias128")
            qwT = pc.tile([128, KT, NL], F16, tag="qwT")
            zmaskT = pc.tile([128, KT], F16, tag="zmaskT")
            magiczT = pc.tile([128, KT], F32, tag="magiczT")
            cnt_row = pc.tile([1, K], F32, tag="cnt_row")
            zmask_row = pc.tile([1, K], F16, tag="zmask_row")

            with tc.tile_pool(name="wpool", bufs=1) as pw:
                # w natural tiles and w.T via xbar — loaded during pass A
                wnat = [pw.tile([128, K], F16, tag=f"wnat{i}", name=f"wnat{i}")
                        for i in range(NW)]
                wT = pw.tile([128, KT, NL], F16, tag="wT")
                for i in range(NW):
                    nc.gpsimd.dma_start(wnat[i][:], w[i * 128:(i + 1) * 128, :])
                for kt in range(KT):
                    nc.sync.dma_start_transpose(
                        out=wT[:, kt, :], in_=w[0:NL, kt * 128:(kt + 1) * 128])

                # ---------------- pass A: activation stats ----------------
                with tc.tile_pool(name="psA", bufs=1,
                                  space=bass.MemorySpace.PSUM) as psA:
                    cnt_ps = [psA.tile([1, 512], F32, tag=f"cnt{b}",
                                       name=f"cnt{b}") for b in range(NB)]
                    with tc.tile_pool(name="pA", bufs=3) as pa:
                        for mt in range(MT):
                            x_t = pa.tile([128, K], F16, tag="x_t")
                            nc.gpsimd.dma_start(
                                x_t[:], x[mt * 128:(mt + 1) * 128, :])
                            abs_t = pa.tile([128, K], F16, tag="abs_t", bufs=2)
                            nc.vector.tensor_scalar(
                                out=abs_t.bitcast(U16)[:],
                                in0=x_t.bitcast(U16)[:],
                                scalar1=0x7FFF, scalar2=None,
                                op0=ALU.bitwise_and)
                            # rowmax via 3 pairwise max folds + small reduce
                            fold = pa.tile([128, K // 2], F16, tag="fold", bufs=1)
                            nc.vector.tensor_max(
                                fold[:, 0:K // 2],
                                abs_t[:, 0:K // 2], abs_t[:, K // 2:K])
                            nc.vector.tensor_max(
                                fold[:, 0:K // 4],
                                fold[:, 0:K // 4], fold[:, K // 4:K // 2])
                            nc.vector.tensor_max(
                                fold[:, 0:K // 8],
                                fold[:, 0:K // 8], fold[:, K // 8:K // 4])
                            nc.vector.tensor_reduce(
                                out=rowmax[:, mt:mt + 1], in_=fold[:, 0:K // 8],
                                axis=mybir.AxisListType.X, op=ALU.max)
                            # relu(|x|-20): column-sum > 0 iff outlier column
                            ind_t = pa.tile([128, K], F16, tag="ind_t", bufs=2)
                            nc.scalar.activation(ind_t[:], abs_t[:], ACTF.Relu,
                                                 bias=neg20[:], scale=1.0)
                            for b in range(NB):
                                nc.tensor.matmul(
                                    cnt_ps[b][:],
                                    onesm_sb[:],
                                    ind_t[:, b * 512:(b + 1) * 512],
                                    start=(mt == 0), stop=(mt == MT - 1))
                    for b in range(NB):
                        nc.vector.tensor_copy(
                            cnt_row[:, b * 512:(b + 1) * 512], cnt_ps[b][:])

                # ------------- finale: masks, scales, q_w' -------------
                nc.vector.tensor_scalar(out=zmask_row[:], in0=cnt_row[:],
                                        scalar1=1e-3, scalar2=None,
                                        op0=ALU.is_lt)
                # zmaskT[p, t] = zmask[t*128+p] via DRAM round-trip
                nc.gpsimd.dma_start(scr_k16[:], zmask_row[0:1, :])
                nc.gpsimd.dma_start(
                    zmaskT[:], scr_k16[:].rearrange("(t p) -> p t", p=128))
                nc.vector.tensor_scalar(out=magiczT[:], in0=zmaskT[:],
                                        scalar1=MAGIC, scalar2=None,
                                        op0=ALU.mult)

                # s_r from rowmax
                inv127 = float(np.float32(1.0) / np.float32(127.0))
                nc.vector.tensor_scalar(out=s_r[:], in0=rowmax[:],
                                        scalar1=inv127, scalar2=1e-8,
                                        op0=ALU.mult, op1=ALU.max)
                nc.vector.reciprocal(s_recip[:], s_r[:])

                with tc.tile_pool(name="psF", bufs=2,
                                  space=bass.MemorySpace.PSUM) as psF:
                    with tc.tile_pool(name="pF", bufs=2) as pf:
                        # zmask128 = ones (x) zmask_row; magicz128 = 1536*z
                        for b in range(NB):
                            bc = psF.tile([128, 512], F32, tag="bc")
                            nc.tensor.matmul(
                                bc[:], ones16_sb[:],
                                zmask_row[:, b * 512:(b + 1) * 512],
                                start=True, stop=True)
                            nc.vector.tensor_copy(
                                zmask128[:, b * 512:(b + 1) * 512], bc[:])
                        nc.vector.tensor_scalar(
                            out=magicz128[:], in0=zmask128[:],
                            scalar1=MAGIC, scalar2=None, op0=ALU.mult)

                        # bias128 = ones (x) bias
                        bcb = psF.tile([128, NL], F32, tag="bcb")
                        nc.tensor.matmul(bcb[:], ones16_sb[:], bias_sb[:],
                                         start=True, stop=True)
                        nc.vector.tensor_copy(bias128[:], bcb[:])

                        # scale_col from natural-layout w
                        wmax = pf.tile([128, NW], F32, tag="wmax")
                        for i in range(NW):
                            wb = pf.tile([128, K], F16, tag="wb")
                            nc.vector.tensor_mul(wb[:], wnat[i][:],
                                                 zmask128[:])
                            nc.vector.tensor_reduce(
                                out=wmax[:, i:i + 1], in_=wb[:],
                                axis=mybir.AxisListType.X, op=ALU.max,
                                apply_absolute_value=True)
                        s_c = pf.tile([128, NW], F32, tag="s_c")
                        s_c16 = pf.tile([128, NW], F16, tag="s_c16")
                        nc.vector.tensor_scalar(out=s_c[:], in0=wmax[:],
                                                scalar1=float(np.float32(1.0) / np.float32(127.0)),
                                                scalar2=None, op0=ALU.mult)
                        nc.vector.tensor_copy(s_c16[:], s_c[:])
                        nc.vector.tensor_copy(s_c[:], s_c16[:])
                        nc.vector.tensor_scalar(out=s_c[:], in0=s_c[:],
                                                scalar1=1e-8, scalar2=None,
                                                op0=ALU.max)
                        s_cr = pf.tile([128, NW], F32, tag="s_cr")
                        nc.vector.reciprocal(s_cr[:], s_c[:])
                        # flatten both to [1, NL] rows via DRAM
                        nc.gpsimd.dma_start(
                            scr_nl[0].rearrange("(t p) -> p t", p=128), s_c[:])
                        nc.gpsimd.dma_start(
                            scr_nl[1].rearrange("(t p) -> p t", p=128), s_cr[:])
                        scol_row = pf.tile([1, NL], F32, tag="scol_row")
                        scr_row = pf.tile([1, NL], F32, tag="scr_row")
                        nc.gpsimd.dma_start(scol_row[0:1, :], scr_nl[0])
                        nc.gpsimd.dma_start(scr_row[0:1, :], scr_nl[1])
                        bc2 = psF.tile([128, NL], F32, tag="bc2")
                        nc.tensor.matmul(bc2[:], ones32_sb[:], scol_row[:],
                                         start=True, stop=True)
                        nc.vector.tensor_copy(scol128[:], bc2[:])
                        bc3 = psF.tile([128, NL], F32, tag="bc2")
                        nc.tensor.matmul(bc3[:], ones32_sb[:], scr_row[:],
                                         start=True, stop=True)
                        nc.vector.tensor_copy(screcipT[:], bc3[:])

                        # q_w' in transposed layout:
                        #   rint(w/s_c) on zmask cols, raw w/s_c on outliers
                        for kt in range(KT):
                            qst = pf.tile([128, NL], F32, tag="qst")
                            nc.vector.tensor_mul(qst[:], wT[:, kt, :],
                                                 screcipT[:])
                            qr = pf.tile([128, NL], F16, tag="qr")
                            nc.scalar.activation(
                                qr[:], qst[:], ACTF.Identity,
                                bias=magiczT[:, kt:kt + 1], scale=1.0)
                            nc.vector.tensor_scalar(
                                out=qwT[:, kt, :], in0=qr[:],
                                scalar1=magiczT[:, kt:kt + 1], scalar2=None,
                                op0=ALU.subtract)

            # ---------------- pass B: quantize + fused GEMM ----------------
            with tc.tile_pool(name="psB", bufs=2,
                              space=bass.MemorySpace.PSUM) as psB:
                with tc.tile_pool(name="pB", bufs=3) as pb:
                    with tc.tile_pool(name="pB2", bufs=2) as pb2:
                        for mt in range(MT):
                            x_t = pb.tile([128, K], F16, tag="x_t")
                            nc.gpsimd.dma_start(
                                x_t[:], x[mt * 128:(mt + 1) * 128, :])
                            # q_b = x/s_r + 1536*zmask  (fp16 out: rint where
                            # zmask==1, raw value on outlier columns)
                            qb = pb2.tile([128, K], F16, tag="qb")
                            nc.vector.scalar_tensor_tensor(
                                out=qb[:], in0=x_t[:],
                                scalar=s_recip[:, mt:mt + 1],
                                in1=magicz128[:],
                                op0=ALU.mult, op1=ALU.add)
                            qt = pb2.tile([128, K], F16, tag="qt")
                            nc.vector.tensor_sub(qt[:], qb[:], magicz128[:])
                            # transpose q to [k, m] via xbar
                            qT = pb2.tile([128, KT, 128], F16, tag="qT")
                            for kt in range(KT):
                                nc.sync.dma_start_transpose(
                                    out=qT[:, kt, :],
                                    in_=qt[:, kt * 128:(kt + 1) * 128])

                            py = psB.tile([128, NL], F32, tag="py")
                            for kt in range(KT):
                                nc.tensor.matmul(
                                    py[:], qT[:, kt, :], qwT[:, kt, :],
                                    start=(kt == 0), stop=(kt == KT - 1))

                            t2e = pb2.tile([128, NL], F32, tag="t2e")
                            nc.vector.scalar_tensor_tensor(
                                out=t2e[:], in0=py[:],
                                scalar=s_r[:, mt:mt + 1], in1=scol128[:],
                                op0=ALU.mult, op1=ALU.mult)
                            y_t = pb2.tile([128, NL], F16, tag="y_t")
                            nc.vector.tensor_add(y_t[:], t2e[:], bias128[:])
                            nc.gpsimd.dma_start(
                                y[mt * 128:(mt + 1) * 128, :], y_t[:])
    if split_waits:
        _split_multi_waits(nc)
    return nc


def make_consts():
    return {
        "ones16": np.ones((1, 128), dtype=np.float16),
        "ones32": np.ones((1, 128), dtype=np.float32),
        "onesm": np.ones((128, 1), dtype=np.float16),
    }


_CACHE = {}


def kernel(x, weight, bias):
    from concourse.bass_utils import run_bass_kernel_spmd

    B, S, K = x.shape
    N = weight.shape[0]
    M = B * S
    NC = 8
    NL = N // NC

    key = (M, K, NL)
    if key not in _CACHE:
        _CACHE[key] = build_bass(M, K, NL)
    nc = _CACHE[key]

    consts = make_consts()
    xf = np.ascontiguousarray(x.reshape(M, K))
    in_maps = []
    for c in range(NC):
        m = dict(consts)
        m["x"] = xf
        m["w"] = np.ascontiguousarray(weight[c * NL:(c + 1) * NL, :])
        m["b"] = np.ascontiguousarray(bias[c * NL:(c + 1) * NL])
        in_maps.append(m)

    global _LAST_NC_INMAPS
    _LAST_NC_INMAPS = (nc, in_maps)
    res = run_bass_kernel_spmd(nc, in_maps, core_ids=list(range(NC)))
    y = np.concatenate([res.results[c]["y"] for c in range(NC)], axis=1)
    return y.reshape(B, S, N).astype(np.float16)



# revision 10
# speedup vs baseline: 4.0907x; 4.0907x over previous
"""MixLinear int8-decomposition GEMM for Trainium2 (8 NeuronCores), v2.

Reference semantics:
  out_mask[k]  = any(|x[:,k]| > 20)                      (outlier columns)
  w_base       = w * (1-mask);  s_c[n] = f16(max|w_base|/127) clamped
  q_w          = rint(w_base / s_c)
  s_r[m]       = max|x[m,:]|/127 clamped
  q_x          = rint(x / s_r)
  y            = (q_x @ q_w.T) * s_r * s_c + (x*mask)@(w*mask).T + bias

Both GEMMs fuse into ONE fp16 PE matmul: for non-outlier columns the
operands are integer-valued q_x and w''=q_w*s_c; for outlier columns the
raw x/s_r and w, so after the (* s_r) epilogue those columns give x*w.
Rounding uses the fp16 +1536 magic bias, applied on the Scalar engine
(fp32 in -> fp16 out Identity activation with per-partition bias
1536*z[k]); the magic offset is NOT subtracted from the operand - its
contribution 1536*(z.w'')[n] is subtracted in the epilogue instead.

v2 structure (vs v1):
  - activation stats pass is sharded 8-ways across cores; partials are
    combined with AllReduce(cnt) + AllGather(s_recip, s_r) collectives.
  - x is loaded K-major via batched 1024-row DMA transposes (256 instrs
    total instead of 2048 128-row ones), quantized in K-major layout.
  - s_c is folded into the weight operand, so the epilogue is 2 vector
    ops and the matmuls stay back-to-back (HAM stays warm).

Sharding: column-parallel over N (out_features) across 8 cores; x replicated.
"""
import sys

sys.path.insert(0, '/opt/trn_rl_repo')

import numpy as np

import concourse.bass as bass
import concourse.mybir as mybir
from concourse.tile import TileContext
from concourse.vector_clock import ScopedClock, VectorClock

F16 = mybir.dt.float16
F32 = mybir.dt.float32
U16 = mybir.dt.uint16
ALU = mybir.AluOpType
ACTF = mybir.ActivationFunctionType

SIGMA = 20.0
MAGIC = 1536.0  # 1.5*2^10: fp16 round-to-nearest-even for |v| <= 127


class SplitDrainTileContext(TileContext):
    """This walrus build rejects >1 sem-wait on CTRL-type (Drain/NOP)
    instructions; pre-drain each clock lane with its own single-wait NOP and
    emit the final drain wait-free."""

    def _drain_and_barrier(self, tick_clock, wait_clock):
        gc = tick_clock.global_clock
        items = gc.items() if hasattr(gc, 'items') else [(None, gc)]
        for scope, vc in items:
            n = len(vc)
            for proc in range(n):
                t = vc[proc]
                if t > 0:
                    vec = [0] * n
                    vec[proc] = t
                    nop = self.nc.sync.nop()
                    wait_clock.add_sem_waits(
                        nop.ins, ScopedClock({scope: VectorClock(vec)}))
        self.nc.sync.drain()
        self.nc.all_engine_barrier()
        assert self.sems is not None
        popped = self.nc._tile_sem_poison_stack.pop()
        assert popped is self._sem_poison
        self.nc.clear_and_free_semaphores(list(self.sems.allocated().values()))
        self.nc.all_engine_barrier()


def _split_multi_waits(nc):
    """This walrus build rejects >1 semaphore wait per instruction (any
    struct). Hoist extra waits onto single-wait NOPs on the same engine,
    placed immediately before the instruction (engines execute their
    instructions in block order, so the stall semantics are identical)."""
    import bass_rust
    counter = [0]
    for f in nc.m.functions:
        for bb in f.blocks:
            new_list = []
            for inst in bb.instructions:
                si = inst.sync_info
                waits = list(si.on_wait) if si is not None else []
                if len(waits) > 1:
                    for wt in waits[:-1]:
                        counter[0] += 1
                        nop = mybir.InstNoOp(
                            name=f"I-waitsplit-{counter[0]}", ins=[], outs=[])
                        nop.engine = inst.engine
                        nop.sync_info = bass_rust.SyncInfo(
                            on_wait=[wt], on_update=[])
                        new_list.append(nop)
                    inst.sync_info = bass_rust.SyncInfo(
                        on_wait=[waits[-1]],
                        on_update=list(si.on_update) if si is not None else [])
                new_list.append(inst)
            bb.instructions = new_list


def build_bass(M, K, NL, C=8, split_waits=True):
    """One-core program: x[M,K]f16 (replicated), xs[M//C,K]f16 (this core's
    row slice), w[NL,K]f16 + bias[NL]f16 (N-shard) -> y[M,NL]f16."""
    MT, KT, NW = M // 128, K // 128, NL // 128
    NB = K // 512          # psum banks for the outlier-count matmuls
    MLOC = M // C          # rows scanned by this core in pass A
    LT = MLOC // 128       # local m-tiles
    BLK = 1024             # pass-B row block (8 m-tiles = 8 psum banks)
    NBLK = M // BLK
    BMT = BLK // 128
    assert NL <= 512 and MLOC == BLK

    nc = bass.Bass(num_devices=C)
    x = nc.declare_dram_parameter("x", [M, K], F16, isOutput=False)
    xs = nc.declare_dram_parameter("xs", [MLOC, K], F16, isOutput=False)
    w = nc.declare_dram_parameter("w", [NL, K], F16, isOutput=False)
    bias = nc.declare_dram_parameter("b", [NL], F16, isOutput=False)
    ones16 = nc.declare_dram_parameter("ones16", [1, 128], F16, isOutput=False)
    ones32 = nc.declare_dram_parameter("ones32", [1, 128], F32, isOutput=False)
    onesm = nc.declare_dram_parameter("onesm", [128, 1], F16, isOutput=False)
    y = nc.declare_dram_parameter("y", [M, NL], F16, isOutput=True)

    scr_k16 = nc.dram_tensor("scr_k16", [K], F16)
    scr_nl = nc.dram_tensor("scr_nl", [2, NL], F32)
    part = nc.dram_tensor("part", [2, MLOC], F32)        # srecip ++ s_r
    gat = nc.dram_tensor("gat", [C, 2, LT, 128], F32, addr_space="Shared")
    cntp = nc.dram_tensor("cntp", [K], F32)
    cntred = nc.dram_tensor("cntred", [K], F32, addr_space="Shared")

    RG = [list(range(C))]
    inv127 = float(np.float32(1.0) / np.float32(127.0))

    with SplitDrainTileContext(nc) as tc:
        with tc.tile_pool(name="const", bufs=1) as pc:
            ones16_sb = pc.tile([1, 128], F16, tag="ones16")
            ones32_sb = pc.tile([1, 128], F32, tag="ones32")
            onesm_sb = pc.tile([128, 1], F16, tag="onesm")
            bias_sb = pc.tile([1, NL], F16, tag="bias")
            nc.gpsimd.dma_start(ones16_sb[:], ones16[:])
            nc.gpsimd.dma_start(ones32_sb[:], ones32[:])
            nc.gpsimd.dma_start(onesm_sb[:], onesm[:])
            nc.gpsimd.dma_start(bias_sb[0:1, :], bias[:])

            neg20 = pc.tile([128, 1], F32, tag="neg20")
            nc.vector.memset(neg20[:], -SIGMA)
            rowmax = pc.tile([128, LT], F32, tag="rowmax")
            zmaskT = pc.tile([128, KT], F16, tag="zmaskT")
            magiczT = pc.tile([128, KT], F32, tag="magiczT")
            scolT16 = pc.tile([128, NL], F16, tag="scolT16")
            screcipT = pc.tile([128, NL], F32, tag="screcipT")
            bias128 = pc.tile([128, NL], F32, tag="bias128")
            czwsc128 = pc.tile([128, NL], F32, tag="czwsc128")
            qwT = pc.tile([128, KT, NL], F16, tag="qwT")
            cnt_row = pc.tile([1, K], F32, tag="cnt_row")
            zmask_row = pc.tile([1, K], F16, tag="zmask_row")

            with tc.tile_pool(name="wpool", bufs=1) as pw:
                # w natural tiles and w.T via xbar - loaded during pass A
                wnat = [pw.tile([128, K], F16, tag=f"wnat{i}", name=f"wnat{i}")
                        for i in range(NW)]
                wT = pw.tile([128, KT, NL], F16, tag="wT")
                for i in range(NW):
                    nc.gpsimd.dma_start(wnat[i][:], w[i * 128:(i + 1) * 128, :])
                for kt in range(KT):
                    nc.sync.dma_start_transpose(
                        out=wT[:, kt, :], in_=w[0:NL, kt * 128:(kt + 1) * 128])

                # ---------- pass A: sharded activation stats ----------
                with tc.tile_pool(name="psA", bufs=1,
                                  space=bass.MemorySpace.PSUM) as psA:
                    cnt_ps = [psA.tile([1, 512], F32, tag=f"cnt{bk}",
                                       name=f"cnt{bk}") for bk in range(NB)]
                    with tc.tile_pool(name="pA", bufs=2) as pa:
                        for lt in range(LT):
                            x_t = pa.tile([128, K], F16, tag="x_t")
                            nc.gpsimd.dma_start(
                                x_t[:], xs[lt * 128:(lt + 1) * 128, :])
                            abs_t = pa.tile([128, K], F16, tag="abs_t")
                            nc.vector.tensor_scalar(
                                out=abs_t.bitcast(U16)[:],
                                in0=x_t.bitcast(U16)[:],
                                scalar1=0x7FFF, scalar2=None,
                                op0=ALU.bitwise_and)
                            fold = pa.tile([128, K // 2], F16, tag="fold",
                                           bufs=1)
                            nc.vector.tensor_max(
                                fold[:, 0:K // 2],
                                abs_t[:, 0:K // 2], abs_t[:, K // 2:K])
                            nc.vector.tensor_max(
                                fold[:, 0:K // 4],
                                fold[:, 0:K // 4], fold[:, K // 4:K // 2])
                            nc.vector.tensor_max(
                                fold[:, 0:K // 8],
                                fold[:, 0:K // 8], fold[:, K // 8:K // 4])
                            nc.vector.tensor_reduce(
                                out=rowmax[:, lt:lt + 1], in_=fold[:, 0:K // 8],
                                axis=mybir.AxisListType.X, op=ALU.max)
                            # relu(|x|-20): column-sum > 0 iff outlier column
                            ind_t = pa.tile([128, K], F16, tag="ind_t")
                            nc.scalar.activation(ind_t[:], abs_t[:], ACTF.Relu,
                                                 bias=neg20[:], scale=1.0)
                            for bk in range(NB):
                                nc.tensor.matmul(
                                    cnt_ps[bk][:],
                                    onesm_sb[:],
                                    ind_t[:, bk * 512:(bk + 1) * 512],
                                    start=(lt == 0), stop=(lt == LT - 1))
                    for bk in range(NB):
                        nc.vector.tensor_copy(
                            cnt_row[:, bk * 512:(bk + 1) * 512], cnt_ps[bk][:])

                # partial counts out -> AllReduce(add) first (finale's long
                # dependency chain hangs off the mask, not off s_r)
                nc.gpsimd.dma_start(cntp[:], cnt_row[0:1, :])
                nc.gpsimd.collective_compute(
                    "AllReduce", ALU.add, replica_groups=RG,
                    ins=[cntp[:].opt()], outs=[cntred[:].opt()])

                # local row scales out -> AllGather
                s_r_loc = pc.tile([128, LT], F32, tag="s_r_loc")
                s_rec_loc = pc.tile([128, LT], F32, tag="s_rec_loc")
                nc.vector.tensor_scalar(out=s_r_loc[:], in0=rowmax[:],
                                        scalar1=inv127, scalar2=1e-8,
                                        op0=ALU.mult, op1=ALU.max)
                nc.vector.reciprocal(s_rec_loc[:], s_r_loc[:])
                nc.gpsimd.dma_start(
                    part[0].rearrange("(t p) -> p t", p=128), s_rec_loc[:])
                nc.gpsimd.dma_start(
                    part[1].rearrange("(t p) -> p t", p=128), s_r_loc[:])
                nc.gpsimd.collective_compute(
                    "AllGather", ALU.bypass, replica_groups=RG,
                    ins=[part[:].opt()], outs=[gat[:].opt()])

                # ---------------- finale: masks, scales, w'' ----------------
                cnt_red_sb = pc.tile([1, K], F32, tag="cnt_red_sb")
                nc.gpsimd.dma_start(cnt_red_sb[0:1, :], cntred[:])
                nc.vector.tensor_scalar(out=zmask_row[:], in0=cnt_red_sb[:],
                                        scalar1=1e-3, scalar2=None,
                                        op0=ALU.is_lt)
                # zmaskT[p, t] = zmask[t*128+p] via DRAM round-trip
                nc.gpsimd.dma_start(scr_k16[:], zmask_row[0:1, :])
                nc.gpsimd.dma_start(
                    zmaskT[:], scr_k16[:].rearrange("(t p) -> p t", p=128))
                nc.vector.tensor_scalar(out=magiczT[:], in0=zmaskT[:],
                                        scalar1=MAGIC, scalar2=None,
                                        op0=ALU.mult)

                with tc.tile_pool(name="psF", bufs=2,
                                  space=bass.MemorySpace.PSUM) as psF:
                    with tc.tile_pool(name="pF", bufs=2) as pf:
                        # zmask128 = ones (x) zmask_row
                        zmask128 = pf.tile([128, K], F16, tag="zmask128",
                                           bufs=1)
                        for bk in range(NB):
                            bc = psF.tile([128, 512], F32, tag="bc")
                            nc.tensor.matmul(
                                bc[:], ones16_sb[:],
                                zmask_row[:, bk * 512:(bk + 1) * 512],
                                start=True, stop=True)
                            nc.vector.tensor_copy(
                                zmask128[:, bk * 512:(bk + 1) * 512], bc[:])

                        # bias128 = ones (x) bias
                        bcb = psF.tile([128, NL], F32, tag="bc")
                        nc.tensor.matmul(bcb[:], ones16_sb[:], bias_sb[:],
                                         start=True, stop=True)
                        nc.vector.tensor_copy(bias128[:], bcb[:])

                        # scale_col from natural-layout w
                        wmax = pf.tile([128, NW], F32, tag="wmax")
                        for i in range(NW):
                            wb = pf.tile([128, K], F16, tag="wb")
                            nc.vector.tensor_mul(wb[:], wnat[i][:],
                                                 zmask128[:])
                            nc.vector.tensor_reduce(
                                out=wmax[:, i:i + 1], in_=wb[:],
                                axis=mybir.AxisListType.X, op=ALU.max,
                                apply_absolute_value=True)
                        s_c = pf.tile([128, NW], F32, tag="s_c")
                        s_c16 = pf.tile([128, NW], F16, tag="s_c16")
                        nc.vector.tensor_scalar(out=s_c[:], in0=wmax[:],
                                                scalar1=inv127,
                                                scalar2=None, op0=ALU.mult)
                        nc.vector.tensor_copy(s_c16[:], s_c[:])
                        nc.vector.tensor_copy(s_c[:], s_c16[:])
                        nc.vector.tensor_scalar(out=s_c[:], in0=s_c[:],
                                                scalar1=1e-8, scalar2=None,
                                                op0=ALU.max)
                        s_cr = pf.tile([128, NW], F32, tag="s_cr")
                        nc.vector.reciprocal(s_cr[:], s_c[:])
                        # flatten both to [1, NL] rows via DRAM
                        nc.gpsimd.dma_start(
                            scr_nl[0].rearrange("(t p) -> p t", p=128), s_c[:])
                        nc.gpsimd.dma_start(
                            scr_nl[1].rearrange("(t p) -> p t", p=128), s_cr[:])
                        scol_row = pf.tile([1, NL], F32, tag="scol_row")
                        scr_row = pf.tile([1, NL], F32, tag="scr_row")
                        nc.gpsimd.dma_start(scol_row[0:1, :], scr_nl[0])
                        nc.gpsimd.dma_start(scr_row[0:1, :], scr_nl[1])
                        bc2 = psF.tile([128, NL], F32, tag="bc")
                        nc.tensor.matmul(bc2[:], ones32_sb[:], scol_row[:],
                                         start=True, stop=True)
                        nc.vector.tensor_copy(scolT16[:], bc2[:])
                        bc3 = psF.tile([128, NL], F32, tag="bc")
                        nc.tensor.matmul(bc3[:], ones32_sb[:], scr_row[:],
                                         start=True, stop=True)
                        nc.vector.tensor_copy(screcipT[:], bc3[:])

                        # w'' in K-major layout:
                        #   z=1 cols: rint(w/s_c)*s_c;  z=0 cols: ~w
                        for kt in range(KT):
                            qst = pf.tile([128, NL], F32, tag="qst")
                            nc.vector.tensor_mul(qst[:], wT[:, kt, :],
                                                 screcipT[:])
                            qr = pf.tile([128, NL], F16, tag="qr")
                            nc.scalar.activation(
                                qr[:], qst[:], ACTF.Identity,
                                bias=magiczT[:, kt:kt + 1], scale=1.0)
                            dq = pf.tile([128, NL], F16, tag="dq")
                            nc.vector.tensor_scalar(
                                out=dq[:], in0=qr[:],
                                scalar1=magiczT[:, kt:kt + 1], scalar2=None,
                                op0=ALU.subtract)
                            nc.vector.tensor_mul(qwT[:, kt, :], dq[:],
                                                 scolT16[:])

                        # czwsc[n] = sum_k z[k] * w''[k,n]  (exact f32)
                        czw_ps = psF.tile([1, NL], F32, tag="czw", bufs=1)
                        for kt in range(KT):
                            nc.tensor.matmul(
                                czw_ps[:], zmaskT[:, kt:kt + 1],
                                qwT[:, kt, :],
                                start=(kt == 0), stop=(kt == KT - 1))
                        czw_row = pf.tile([1, NL], F32, tag="czw_row")
                        nc.vector.tensor_copy(czw_row[:], czw_ps[:])
                        bc4 = psF.tile([128, NL], F32, tag="bc")
                        nc.tensor.matmul(bc4[:], ones32_sb[:], czw_row[:],
                                         start=True, stop=True)
                        nc.vector.tensor_copy(czwsc128[:], bc4[:])

            # ---------------- pass B: quantize + fused GEMM ----------------
            with tc.tile_pool(name="pB", bufs=1) as pb:
                # broadcast row scales from the AllGather layout
                s_r_all = pb.tile([128, MT], F32, tag="s_r_all")
                srecip_ball = pb.tile([128, M], F32, tag="srecip_ball")
                with tc.tile_pool(name="psG", bufs=2,
                                  space=bass.MemorySpace.PSUM) as psG:
                    with tc.tile_pool(name="pG", bufs=2) as pg:
                        for b in range(NBLK):
                            srow = pg.tile([1, BLK], F32, tag="srow")
                            nc.gpsimd.dma_start(
                                srow[0:1, :], gat[b, 0, :, :].opt())
                            for h in range(BLK // 512):
                                bc = psG.tile([128, 512], F32, tag="bc")
                                nc.tensor.matmul(
                                    bc[:], ones32_sb[:],
                                    srow[:, h * 512:(h + 1) * 512],
                                    start=True, stop=True)
                                nc.vector.tensor_copy(
                                    srecip_ball[:, b * BLK + h * 512:
                                                b * BLK + (h + 1) * 512],
                                    bc[:])
                        for c in range(C):
                            nc.gpsimd.dma_start(
                                s_r_all[:, c * LT:(c + 1) * LT],
                                gat[c, 1, :, :].rearrange("t p -> p t"))

                with tc.tile_pool(name="psB", bufs=1,
                                  space=bass.MemorySpace.PSUM) as psB:
                    py = [psB.tile([128, NL], F32, tag=f"py{mt}",
                                   name=f"py{mt}") for mt in range(BMT)]
                    with tc.tile_pool(name="pBi", bufs=6) as pbi:
                        for b in range(NBLK):
                            for kt in range(KT):
                                xT_sl = pbi.tile([128, BLK], F16, tag="xT_sl")
                                nc.sync.dma_start_transpose(
                                    out=xT_sl[:],
                                    in_=x[b * BLK:(b + 1) * BLK,
                                          kt * 128:(kt + 1) * 128])
                                t32 = pbi.tile([128, BLK], F32, tag="t32",
                                               bufs=3)
                                nc.vector.tensor_mul(
                                    t32[:], xT_sl[:],
                                    srecip_ball[:, b * BLK:(b + 1) * BLK])
                                qs = pbi.tile([128, BLK], F16, tag="qs")
                                nc.scalar.activation(
                                    qs[:], t32[:], ACTF.Identity,
                                    bias=magiczT[:, kt:kt + 1], scale=1.0)
                                for mt in range(BMT):
                                    nc.tensor.matmul(
                                        py[mt][:],
                                        qs[:, mt * 128:(mt + 1) * 128],
                                        qwT[:, kt, :],
                                        start=(kt == 0), stop=(kt == KT - 1))
                            for mt in range(BMT):
                                g = b * BMT + mt
                                t_ep = pbi.tile([128, NL], F32, tag="t_ep",
                                                bufs=3)
                                nc.vector.scalar_tensor_tensor(
                                    out=t_ep[:], in0=czwsc128[:],
                                    scalar=-MAGIC, in1=py[mt][:],
                                    op0=ALU.mult, op1=ALU.add)
                                y_t = pbi.tile([128, NL], F16, tag="y_t",
                                               bufs=3)
                                nc.vector.scalar_tensor_tensor(
                                    out=y_t[:], in0=t_ep[:],
                                    scalar=s_r_all[:, g:g + 1],
                                    in1=bias128[:],
                                    op0=ALU.mult, op1=ALU.add)
                                nc.gpsimd.dma_start(
                                    y[g * 128:(g + 1) * 128, :], y_t[:])
    if split_waits:
        _split_multi_waits(nc)
    return nc


def make_consts():
    return {
        "ones16": np.ones((1, 128), dtype=np.float16),
        "ones32": np.ones((1, 128), dtype=np.float32),
        "onesm": np.ones((128, 1), dtype=np.float16),
    }


_CACHE = {}


def _build_inmaps(x, weight, bias):
    B, S, K = x.shape
    N = weight.shape[0]
    M = B * S
    NC = 8
    NL = N // NC
    MLOC = M // NC
    consts = make_consts()
    xf = np.ascontiguousarray(x.reshape(M, K))
    in_maps = []
    for c in range(NC):
        m = dict(consts)
        m["x"] = xf
        m["xs"] = np.ascontiguousarray(xf[c * MLOC:(c + 1) * MLOC])
        m["w"] = np.ascontiguousarray(weight[c * NL:(c + 1) * NL, :])
        m["b"] = np.ascontiguousarray(bias[c * NL:(c + 1) * NL])
        in_maps.append(m)
    return (M, K, NL), in_maps


def kernel(x, weight, bias):
    from concourse.bass_utils import run_bass_kernel_spmd

    B, S, K = x.shape
    N = weight.shape[0]
    NC = 8

    key, in_maps = _build_inmaps(x, weight, bias)
    if key not in _CACHE:
        _CACHE[key] = build_bass(*key, C=NC)
    nc = _CACHE[key]

    global _LAST_NC_INMAPS
    _LAST_NC_INMAPS = (nc, in_maps)
    res = run_bass_kernel_spmd(nc, in_maps, core_ids=list(range(NC)))
    NL = N // NC
    y = np.concatenate([res.results[c]["y"] for c in range(NC)], axis=1)
    return y.reshape(B, S, N).astype(np.float16)
